# revision 36
# baseline (speedup 1.0000x reference)
"""Trainium2 Bass kernel for nn_Diff_Label01_Loss (masked cosine-similarity loss).

Contract: kernel(labels, datas) takes FULL inputs (labels [8192,2] f32,
datas [8192,4096] f32), returns (total_loss, sim_loss, differ_loss).

Strategy — shard D (columns) across the 8 cores; NO collective:
  Core c owns cols [c*512, (c+1)*512) of datas, in TWO fp8 layouts
  (8.4MB/core total):
    x_rm [128, 64, 512]   row-tiles (partition p of tile t = row t*128+p)
    xT   [128, 2, 2, 8, 2, 512]  [p, g, h, j, kt, c] = x[h*4096+j*512+c,
                                  (2g+kt)*128+p] — pair-interleaved for
                                  DoubleRow fp8 matmuls
  s0_c   = masked column sum of the core's slice — PE DoubleRow matmuls,
           mask-pair stationary, accumulated in psum[0:1, 0:512]
  m8_c   = fp8(bf16(s0_c * 2^-6)) — ACT cast to bf16, four K=1 matmuls
           spread it onto partitions, ACT copies into pair-layout slots
  numer  = x_slice @ m8_c — PE DoubleRow into psum[0:1, 0:4096]; half B
           reuses the row after half A's spill (DR matmuls to psum
           partition 32/64 fail ISA checks); [1,2048] spill pieces split
           across DVE and ACT
  normsq = per-row sum of squares — split DVE scalar_tensor_tensor /
           ACT Square activation, 38/26 tiles, chasing the x_rm ingest
           chunks in arrival order (GPSIMD cannot run tensor ops on this
           compiler, and fp8 has no 2x DVE mode, so these two engines are
           the throughput wall of the whole kernel)

  The PE runs junk matmuls on garbage SBUF during the ~10us DMA-issue /
  preamble window so the HAM clock gate is already at full rate when the
  real DoubleRow stream starts; a few more keep it warm across the
  m-dance gap. DMA completion semaphores lag the last data byte by
  several us when both HWDGE rings are busy (per-engine sem-inc
  descriptors pay the write-receipt round trip), which is why the chunk
  schedule leans early.

Host: packs fp8 layouts, then combines per-core partials in f64:
  numer_i = sum_c numer_c[i]; |x_i|^2 = sum_c normsq_c[i];
  |m|^2 = sum_c |m8_c|^2; cos_i = numer_i / (|x_i| |m|) — scale-invariant
  in m, so the 2^-6 scaling and the n0 division drop out.
"""

import contextlib

import numpy as np

B = 8192
D = 4096
P = 128
NCORES = 8
DC = D // NCORES        # 512 cols per core
T = B // P              # 64 row tiles
NK = DC // P            # 4 col chunks
HB = B // 2             # rows per half
MS = 2.0 ** -6          # m scale (keeps s0 in fp8 range)
EPS = 1e-8
# x_rm ingest chunks: (lo, hi, fused_dve_tiles, act_group_starts).
# Fused tiles run as one DVE scalar_tensor_tensor+accum each; group tiles are
# squared 4-at-a-time by ACT into bf16 scratch (skipping the 278ns
# ACTIVATION_READ_ACCUMULATOR tax per tile) and summed by DVE with one
# multi-group tensor_reduce per group (bf16 single-src hits the fast mode).
CHUNKS = [
    (0, 4, [], [0]),                           # ch0: 0.25MB, SP ring
    (4, 16, [4, 5, 6, 7], [8, 12]),            # ch1: 0.75MB, SP
    (16, 32, [16, 17, 18, 19], [20, 24, 28]),  # ch2: 1MB, SWDGE ring
    (32, 48, [32, 33, 34, 35], [36, 40, 44]),  # ch3: 1MB, SP
    (48, 64, [48, 49, 50, 51], [52, 56, 60]),  # ch4: 1MB, ACT ring
]
NGRP = sum(len(c[3]) for c in CHUNKS)          # 12 groups = 48 tiles
NV_TOT = sum(len(c[2]) for c in CHUNKS) + NGRP  # DVE incs: 16 fused + 12 reduces
# ACT executes groups in this order (chasing chunk arrival); DVE reduces in
# the same order, gating on the s_sq counter.
GSCHED = [0, 52, 56, 60, 8, 12, 20, 24, 28, 36, 40, 44]


def _build_program():
    import concourse.bass as bass
    import concourse.mybir as mybir

    f32 = mybir.dt.float32
    bf16 = mybir.dt.bfloat16
    fp8 = mybir.dt.float8e4
    AOP = mybir.AluOpType
    AF = mybir.ActivationFunctionType
    DR = mybir.MatmulPerfMode.DoubleRow

    nc = bass.Bass(trn_type="TRN2", num_devices=NCORES)

    xrm_d = nc.dram_tensor("xrm", [P, T * DC], fp8, kind="ExternalInput")
    xt_d = nc.dram_tensor("xt", [P, NK * B], fp8, kind="ExternalInput")
    m0_d = nc.dram_tensor("m0", [P, T], fp8, kind="ExternalInput")
    out_num = nc.dram_tensor("out_num", [1, B], f32, kind="ExternalOutput")
    out_nrm = nc.dram_tensor("out_nrm", [P, T], f32, kind="ExternalOutput")
    out_m8p = nc.dram_tensor("out_m8p", [P, 128], fp8, kind="ExternalOutput")

    ctx = contextlib.ExitStack()
    sb = lambda name, shape, dt: ctx.enter_context(nc.sbuf_tensor(name, shape, dt))

    x_rm = sb("x_rm", [P, T * DC], fp8)
    xts = sb("xts", [P, NK * B], fp8)
    m0s = sb("m0s", [P, T], fp8)         # [p, a*32+t2] = mask0(row (2*t2+a)*128+p)
    m8pad = sb("m8pad", [P, 128], fp8)   # stationary slots: col k*32 = m[k*128+p]
    dumpV = sb("dumpV", [P, 1], fp8)
    dumpA = sb("dumpA", [P, 1], fp8)
    normsq = sb("normsq", [P, T], f32)
    m16row = sb("m16row", [1, DC], bf16)
    one1 = sb("one1", [1, 1], bf16)
    nsp = sb("nsp", [1, B], f32)         # numer row
    xsq = sb("xsq", [P, 12 * 4 * DC], bf16)  # squared-tile scratch, 1 slot/group
    junkb = sb("junkb", [1, 1024], bf16)  # never written; junk warmup reads

    pt = ctx.enter_context(nc.psum_tensor("pt", [P, 4096]))

    sem = lambda name: ctx.enter_context(nc.semaphore(name))
    dxr = [sem(f"dxr{i}") for i in range(len(CHUNKS))]
    dxt = {(h, g): sem(f"dxt{h}{g}") for h in range(2) for g in range(2)}
    sm0 = sem("sm0")
    s_pe = sem("s_pe")
    s_cast = sem("s_cast")
    s_tr = sem("s_tr")
    s_m8 = sem("s_m8")
    s_hA = sem("s_hA")
    s_hB = sem("s_hB")
    s_spA = sem("s_spA")
    s_spB = sem("s_spB")
    s_nsV = sem("s_nsV")
    s_sq = sem("s_sq")
    s_out = sem("s_out")

    xrm3 = x_rm.rearrange("p (t c) -> p t c", c=DC)
    xsq4 = xsq.rearrange("p (g k c) -> p g k c", k=4, c=DC)
    xt6 = xts.rearrange("p (g h j k c) -> p g h j k c", g=2, h=2, j=8, k=2)
    m0d = m0s.rearrange("p (a t) -> p a t", a=2)

    def rm_chunk(q):
        lo, hi = CHUNKS[q][0], CHUNKS[q][1]
        return slice(lo * DC, hi * DC)

    def xt_sl(h, g):
        base = (g * 2 + h) * HB * 2
        return slice(base, base + HB * 2)   # full 1MB block

    gslot = {s: i for i, s in enumerate(GSCHED)}  # group start tile -> scratch slot

    with nc.Block() as block:

        @block.sync
        def _(sync):
            sync.dma_start(m0s[:, :], m0_d[:, :]).then_inc(sm0, 16)
            for q in (0, 1, 3):
                sl = rm_chunk(q)
                sync.dma_start(x_rm[:, sl], xrm_d[:, sl]).then_inc(dxr[q], 16)
            for (h, g) in ((0, 0), (1, 0)):
                sl = xt_sl(h, g)
                sync.dma_start(xts[:, sl], xt_d[:, sl]).then_inc(dxt[(h, g)], 16)
            # numer half A out as soon as both spill pieces land
            sync.wait_ge(s_spA, 2)
            sync.dma_start(out_num[:, 0:HB], nsp[:, 0:HB]).then_inc(s_out, 16)
            sync.wait_ge(s_spB, 2)
            sync.dma_start(out_num[:, HB:B], nsp[:, HB:B]).then_inc(s_out, 16)
            # normsq out once DVE has written all fused + reduced columns
            sync.wait_ge(s_nsV, NV_TOT)
            sync.dma_start(out_nrm[:, :], normsq[:, :]).then_inc(s_out, 16)
            sync.wait_ge(s_out, 64)

        @block.scalar
        def _(sc):
            sc.dma_start(x_rm[:, rm_chunk(4)], xrm_d[:, rm_chunk(4)]).then_inc(dxr[4], 16)
            for (h, g) in ((0, 1), (1, 1)):
                sl = xt_sl(h, g)
                sc.dma_start(xts[:, sl], xt_d[:, sl]).then_inc(dxt[(h, g)], 16)

            def square(s):
                sc.activation(xsq4[:, gslot[s]], xrm3[:, s : s + 4, :],
                              AF.Square).then_inc(s_sq, 1)

            # squares chase chunk arrival: ch0, ch4, ch1[0] before the dance
            sc.wait_ge(dxr[0], 16)
            square(0)
            sc.wait_ge(dxr[4], 16)
            for s in (52, 56, 60):
                square(s)
            sc.wait_ge(dxr[1], 16)
            square(8)
            # m dance: cast s0 -> bf16 row; after PE spreads it, pack fp8 slots
            sc.wait_ge(s_pe, 1)
            sc.activation(m16row[:, :], pt[0:1, 0:DC], AF.Copy, scale=MS).then_inc(s_cast, 1)
            sc.wait_ge(s_tr, 1)
            sc.copy(m8pad[:, 0:97:32], pt[:, 4092:4096]).then_inc(s_m8, 1)
            sc.dma_start(out_m8p[:, :], m8pad[:, :]).then_inc(s_out, 16)
            square(12)
            sc.wait_ge(dxr[2], 16)
            for s in (20, 24):
                square(s)
            # numer half A spill piece (DVE takes the other half)
            sc.wait_ge(s_hA, 1)
            sc.copy(nsp[0:1, 2048:HB], pt[0:1, 2048:4096]).then_inc(s_spA, 1)
            square(28)
            sc.wait_ge(dxr[3], 16)
            for s in (36, 40):
                square(s)
            sc.wait_ge(s_hB, 1)
            sc.copy(nsp[0:1, HB + 2048 : B], pt[0:1, 2048:4096]).then_inc(s_spB, 1)
            square(44)

        @block.vector
        def _(ve):
            def fused(t):
                nc.vector.scalar_tensor_tensor(
                    dumpV[:, 0:1].to_broadcast((P, DC)), xrm3[:, t, :], 1.0,
                    xrm3[:, t, :], AOP.mult, AOP.mult,
                    accum_out=normsq[:, t : t + 1],
                ).then_inc(s_nsV, 1)

            def reduce(s):
                ve.wait_ge(s_sq, GSCHED.index(s) + 1)
                nc.vector.tensor_reduce(
                    normsq[:, s : s + 4], xsq4[:, gslot[s]],
                    axis=mybir.AxisListType.X, op=AOP.add,
                ).then_inc(s_nsV, 1)

            reduce(0)
            ve.wait_ge(dxr[1], 16)
            for t in CHUNKS[1][2]:
                fused(t)
            for s in (52, 56, 60, 8):
                reduce(s)
            ve.wait_ge(dxr[2], 16)
            for t in CHUNKS[2][2]:
                fused(t)
            for s in (12, 20):
                reduce(s)
            ve.wait_ge(dxr[3], 16)
            for t in CHUNKS[3][2]:
                fused(t)
            # numer half A spill piece; ACT takes the other half
            ve.wait_ge(s_hA, 1)
            nc.vector.tensor_copy(nsp[0:1, 0:2048], pt[0:1, 0:2048]).then_inc(s_spA, 1)
            for s in (24, 28):
                reduce(s)
            ve.wait_ge(dxr[4], 16)
            for t in CHUNKS[4][2]:
                fused(t)
            for s in (36,):
                reduce(s)
            # numer half B spill piece
            ve.wait_ge(s_hB, 1)
            nc.vector.tensor_copy(nsp[0:1, HB : HB + 2048], pt[0:1, 0:2048]).then_inc(s_spB, 1)
            for s in (40, 44):
                reduce(s)
        @block.gpsimd
        def _(gp):
            gp.memset(one1[:, :], 1.0)
            gp.dma_start(x_rm[:, rm_chunk(2)], xrm_d[:, rm_chunk(2)]).then_inc(dxr[2], 16)

        @block.tensor
        def _(pe):
            # HAM warmup: junk matmuls on garbage SBUF while DMA issues/preamble
            # run; keeps the PE clock gate at full rate for the real stream.
            for _ in range(16):
                nc.tensor.matmul(
                    pt[64:65, 0:256], junkb[0:1, 0:1], junkb[0:1, 0:256],
                    start=True, stop=True,
                )
            # s0: DoubleRow over row-tile pairs -> psum[0:1, 0:512]
            pe.wait_ge(sm0, 16)
            s0_order = [0, 4, 1, 2, 3]
            first = True
            for ci in s0_order:
                lo, hi = CHUNKS[ci][0], CHUNKS[ci][1]
                pe.wait_ge(dxr[ci], 16)
                for t2 in range(lo // 2, hi // 2):
                    mm = nc.tensor.matmul(
                        pt[0:1, 0:DC],
                        m0d[:, :, t2 : t2 + 1],
                        xrm3[:, 2 * t2 : 2 * t2 + 2, :],
                        start=first, stop=(ci == 3 and t2 == hi // 2 - 1),
                        perf_mode=DR,
                    )
                    first = False
            mm.then_inc(s_pe, 1)
            # spread m16row chunks onto partitions: K=1 matmuls vs ones
            pe.wait_ge(s_cast, 1)
            for k in range(NK):
                mm = nc.tensor.matmul(
                    pt[:, 4092 + k : 4093 + k],
                    m16row[0:1, k * P : (k + 1) * P],
                    one1[0:1, 0:1],
                    start=True, stop=True,
                )
            mm.then_inc(s_tr, 1)
            # keep the PE clock warm while waiting for m8pad + xt arrival
            for _ in range(4):
                nc.tensor.matmul(
                    pt[64:65, 0:256], junkb[0:1, 0:1], junkb[0:1, 0:256],
                    start=True, stop=True,
                )
            pe.wait_ge(s_m8, 1)
            # numer: DoubleRow into psum[0:1, :]; half B reuses the same psum
            # row, so it waits until both half A spill pieces are out.
            # g-outer order so the two contraction halves accumulate per column.
            for h in range(2):
                pe.wait_ge(dxt[(h, 0)], 16)
                pe.wait_ge(dxt[(h, 1)], 16)
                if h == 1:
                    pe.wait_ge(s_spA, 2)
                for g in range(2):
                    for j in range(8):
                        mm = nc.tensor.matmul(
                            pt[0:1, j * DC : (j + 1) * DC],
                            m8pad[:, g * 64 : g * 64 + 33 : 32],
                            xt6[:, g, h, j, :, :],
                            start=(g == 0), stop=(g == 1),
                            perf_mode=DR,
                        )
                mm.then_inc(s_hA if h == 0 else s_hB, 1)

    ctx.close()
    return nc


_PROGRAM = None
LAST_RESULT = None  # BassKernelResults of the most recent run (for profiling)


def _host_inputs(labels, datas):
    import ml_dtypes

    fp8 = ml_dtypes.float8_e4m3
    labels = np.asarray(labels, dtype=np.float32)
    datas = np.asarray(datas, dtype=np.float32)

    mask0 = (labels[:, 0] >= labels[:, 1]).astype(np.float32)  # argmax==0
    x8 = datas.astype(fp8)

    # m0 pair layout: [p, a*32+t2] = mask0[(2*t2+a)*128+p]
    mt = mask0.reshape(T, P)
    m0 = np.empty((P, T), dtype=np.float32)
    half = T // 2
    m0[:, 0:half] = mt[0::2].T
    m0[:, half:T] = mt[1::2].T
    m0 = np.ascontiguousarray(m0).astype(fp8)

    in_maps = []
    for c in range(NCORES):
        xc = x8[:, c * DC : (c + 1) * DC]                       # [8192, 512] fp8
        x_rm = np.ascontiguousarray(
            xc.reshape(T, P, DC).transpose(1, 0, 2)).reshape(P, T * DC)
        xt = np.ascontiguousarray(
            xc.T.reshape(2, 2, P, 2, 8, 512).transpose(2, 0, 3, 4, 1, 5)
        ).reshape(P, NK * B)
        in_maps.append({"xrm": x_rm, "xt": xt, "m0": m0})
    return in_maps, mask0


def _host_finish(results, mask0):
    mask0 = mask0.astype(np.float64)
    mask1 = 1.0 - mask0
    n0 = float(mask0.sum())
    n1 = float(mask1.sum())

    numer = np.zeros(B)
    normsq = np.zeros(B)
    msq = 0.0
    for c in range(NCORES):
        r = results[c]
        numer += np.asarray(r["out_num"], dtype=np.float64).reshape(-1)
        normsq += np.asarray(r["out_nrm"], dtype=np.float64).T.reshape(-1)
        m8p = np.asarray(r["out_m8p"]).astype(np.float64)
        for k in range(NK):
            msq += float((m8p[:, k * 32] ** 2).sum())

    if n0 > 0.0:
        xnorm = np.maximum(np.sqrt(normsq), EPS)
        mnorm = max(np.sqrt(msq), EPS * MS * max(n0, 1.0))
        q = np.abs(numer) / (xnorm * mnorm)
        sim = 1.0 - float((mask0 * q).sum()) / n0
        dif = float((mask1 * q).sum()) / n1 if n1 > 0.0 else 0.0
    else:
        sim = 0.0
        dif = 0.0

    sim = np.float32(sim)
    dif = np.float32(dif)
    return (np.float32(sim + dif), sim, dif)


def kernel(labels, datas):
    global _PROGRAM, LAST_RESULT
    from concourse.bass_utils import run_bass_kernel_spmd

    in_maps, mask0 = _host_inputs(labels, datas)
    if _PROGRAM is None:
        _PROGRAM = _build_program()
    res = run_bass_kernel_spmd(_PROGRAM, in_maps, list(range(NCORES)))
    LAST_RESULT = res
    return _host_finish(res.results, mask0)


# revision 37
# speedup vs baseline: 1.3775x; 1.3775x over previous
"""Trainium2 Bass kernel for nn_Diff_Label01_Loss (masked cosine-similarity loss).

Contract: kernel(labels, datas) takes FULL inputs (labels [8192,2] f32,
datas [8192,4096] f32), returns (total_loss, sim_loss, differ_loss).

Strategy — shard D (columns) across the 8 cores; NO collective:
  Core c owns cols [c*512, (c+1)*512) of datas, in TWO fp8 layouts
  (8.4MB/core total):
    x_rm [128, 64, 512]   row-tiles (partition p of tile t = row t*128+p)
    xT   [128, 2, 2, 8, 2, 512]  [p, g, h, j, kt, c] = x[h*4096+j*512+c,
                                  (2g+kt)*128+p] — pair-interleaved for
                                  DoubleRow fp8 matmuls
  s0_c   = masked column sum of the core's slice — PE DoubleRow matmuls,
           mask-pair stationary, accumulated in psum[0:1, 0:512]
  m8_c   = fp8(bf16(s0_c * 2^-6)) — ACT cast to bf16, four K=1 matmuls
           spread it onto partitions, ACT copies into pair-layout slots
  numer  = x_slice @ m8_c — PE DoubleRow into psum[0:1, 0:4096]; half B
           reuses the row after half A's spill (DR matmuls to psum
           partition 32/64 fail ISA checks); [1,2048] spill pieces split
           across DVE and ACT
  normsq = per-row sum of squares — split DVE scalar_tensor_tensor /
           ACT Square activation, 38/26 tiles, chasing the x_rm ingest
           chunks in arrival order (GPSIMD cannot run tensor ops on this
           compiler, and fp8 has no 2x DVE mode, so these two engines are
           the throughput wall of the whole kernel)

  The PE runs junk matmuls on garbage SBUF during the ~10us DMA-issue /
  preamble window so the HAM clock gate is already at full rate when the
  real DoubleRow stream starts; a few more keep it warm across the
  m-dance gap. DMA completion semaphores lag the last data byte by
  several us when both HWDGE rings are busy (per-engine sem-inc
  descriptors pay the write-receipt round trip), which is why the chunk
  schedule leans early.

Host: packs fp8 layouts, then combines per-core partials in f64:
  numer_i = sum_c numer_c[i]; |x_i|^2 = sum_c normsq_c[i];
  |m|^2 = sum_c |m8_c|^2; cos_i = numer_i / (|x_i| |m|) — scale-invariant
  in m, so the 2^-6 scaling and the n0 division drop out.
"""

import contextlib

import numpy as np

B = 8192
D = 4096
P = 128
NCORES = 8
DC = D // NCORES        # 512 cols per core
T = B // P              # 64 row tiles
NK = DC // P            # 4 col chunks
HB = B // 2             # rows per half
MS = 2.0 ** -6          # m scale (keeps s0 in fp8 range)
EPS = 1e-8
# x_rm ingest chunks: tile ranges [lo, hi) and (DVE, ACT) normsq split.
# Chunk 0 is split 4/12 tiles so the vector engines start ~2us earlier.
CHUNKS = [
    (0, 4, (3, 1)),      # 0a: 0.25MB, SP ring first
    (4, 16, (7, 5)),     # 0b: 0.75MB, SP
    (16, 32, (10, 6)),   # 1:  1MB, ACT ring
    (32, 48, (9, 7)),    # 2:  1MB, SP
    (48, 64, (9, 7)),    # 3:  1MB, ACT
]
NV_TOT = sum(c[2][0] for c in CHUNKS)
NA_TOT = sum(c[2][1] for c in CHUNKS)


def _build_program():
    import concourse.bass as bass
    import concourse.mybir as mybir

    f32 = mybir.dt.float32
    bf16 = mybir.dt.bfloat16
    fp8 = mybir.dt.float8e4
    AOP = mybir.AluOpType
    AF = mybir.ActivationFunctionType
    DR = mybir.MatmulPerfMode.DoubleRow

    nc = bass.Bass(trn_type="TRN2", num_devices=NCORES)

    xrm_d = nc.dram_tensor("xrm", [P, T * DC], fp8, kind="ExternalInput")
    xt_d = nc.dram_tensor("xt", [P, NK * B], fp8, kind="ExternalInput")
    m0_d = nc.dram_tensor("m0", [P, T], fp8, kind="ExternalInput")
    out_num = nc.dram_tensor("out_num", [1, B], f32, kind="ExternalOutput")
    out_nrm = nc.dram_tensor("out_nrm", [P, T], f32, kind="ExternalOutput")
    out_m8p = nc.dram_tensor("out_m8p", [P, 128], fp8, kind="ExternalOutput")

    ctx = contextlib.ExitStack()
    sb = lambda name, shape, dt: ctx.enter_context(nc.sbuf_tensor(name, shape, dt))

    x_rm = sb("x_rm", [P, T * DC], fp8)
    xts = sb("xts", [P, NK * B], fp8)
    m0s = sb("m0s", [P, T], fp8)         # [p, a*32+t2] = mask0(row (2*t2+a)*128+p)
    m8pad = sb("m8pad", [P, 128], fp8)   # stationary slots: col k*32 = m[k*128+p]
    dumpV = sb("dumpV", [P, 1], fp8)
    dumpA = sb("dumpA", [P, 1], fp8)
    normsq = sb("normsq", [P, T], f32)
    m16row = sb("m16row", [1, DC], bf16)
    one1 = sb("one1", [1, 1], bf16)
    nsp = sb("nsp", [1, B], f32)         # numer row
    junkb = sb("junkb", [1, 1024], bf16)  # never written; junk warmup reads

    pt = ctx.enter_context(nc.psum_tensor("pt", [P, 4096]))

    sem = lambda name: ctx.enter_context(nc.semaphore(name))
    dxr = [sem(f"dxr{i}") for i in range(len(CHUNKS))]
    dxt = {(h, g): sem(f"dxt{h}{g}") for h in range(2) for g in range(2)}
    sm0 = sem("sm0")
    s_pe = sem("s_pe")
    s_cast = sem("s_cast")
    s_tr = sem("s_tr")
    s_m8 = sem("s_m8")
    s_hA = sem("s_hA")
    s_hB = sem("s_hB")
    s_spA = sem("s_spA")
    s_spB = sem("s_spB")
    s_nsV = sem("s_nsV")
    s_nsA = sem("s_nsA")
    s_out = sem("s_out")

    xrm3 = x_rm.rearrange("p (t c) -> p t c", c=DC)
    xt6 = xts.rearrange("p (g h j k c) -> p g h j k c", g=2, h=2, j=8, k=2)
    m0d = m0s.rearrange("p (a t) -> p a t", a=2)

    def rm_chunk(q):
        lo, hi = CHUNKS[q][0], CHUNKS[q][1]
        return slice(lo * DC, hi * DC)

    def xt_sl(h, g):
        base = (g * 2 + h) * HB * 2
        return slice(base, base + HB * 2)   # full 1MB block

    # tile ownership within a chunk: DVE first, ACT rest
    def tiles_of(c, eng):
        lo, hi, (nv, na) = CHUNKS[c]
        if eng == "V":
            return range(lo, lo + nv)
        return range(lo + nv, hi)

    with nc.Block() as block:

        @block.sync
        def _(sync):
            sync.dma_start(m0s[:, :], m0_d[:, :]).then_inc(sm0, 16)
            for q in (0, 1, 3):
                sl = rm_chunk(q)
                sync.dma_start(x_rm[:, sl], xrm_d[:, sl]).then_inc(dxr[q], 16)
            for (h, g) in ((0, 0), (1, 0)):
                sl = xt_sl(h, g)
                sync.dma_start(xts[:, sl], xt_d[:, sl]).then_inc(dxt[(h, g)], 16)
            # numer half A out as soon as both spill pieces land
            sync.wait_ge(s_spA, 2)
            sync.dma_start(out_num[:, 0:HB], nsp[:, 0:HB]).then_inc(s_out, 16)
            sync.wait_ge(s_spB, 2)
            sync.dma_start(out_num[:, HB:B], nsp[:, HB:B]).then_inc(s_out, 16)
            # normsq out when both engines are done
            sync.wait_ge(s_nsV, NV_TOT)
            sync.wait_ge(s_nsA, NA_TOT)
            sync.dma_start(out_nrm[:, :], normsq[:, :]).then_inc(s_out, 16)
            sync.wait_ge(s_out, 64)

        @block.scalar
        def _(sc):
            for q, sem_ in ((2, dxr[2]), (4, dxr[4])):
                sl = rm_chunk(q)
                sc.dma_start(x_rm[:, sl], xrm_d[:, sl]).then_inc(sem_, 16)
            for (h, g) in ((0, 1), (1, 1)):
                sl = xt_sl(h, g)
                sc.dma_start(xts[:, sl], xt_d[:, sl]).then_inc(dxt[(h, g)], 16)

            def act_tile(t):
                sc.activation(dumpA[:, 0:1].to_broadcast((P, DC)), xrm3[:, t, :],
                              AF.Square,
                              accum_out=normsq[:, t : t + 1]).then_inc(s_nsA, 1)

            # 11 tiles before the cast (lands ~when s0 completes)
            sc.wait_ge(dxr[0], 16)
            for t in tiles_of(0, "A"):
                act_tile(t)
            sc.wait_ge(dxr[1], 16)
            for t in tiles_of(1, "A"):
                act_tile(t)
            sc.wait_ge(dxr[2], 16)
            for t in list(tiles_of(2, "A"))[:5]:
                act_tile(t)
            # m dance: cast s0 -> bf16 row; after PE spreads it, pack fp8 slots
            sc.wait_ge(s_pe, 1)
            sc.activation(m16row[:, :], pt[0:1, 0:DC], AF.Copy, scale=MS).then_inc(s_cast, 1)
            sc.wait_ge(s_tr, 1)
            sc.copy(m8pad[:, 0:97:32], pt[:, 4092:4096]).then_inc(s_m8, 1)
            sc.dma_start(out_m8p[:, :], m8pad[:, :]).then_inc(s_out, 16)
            for t in list(tiles_of(2, "A"))[5:]:
                act_tile(t)
            sc.wait_ge(dxr[4], 16)
            for t in list(tiles_of(4, "A"))[:3]:
                act_tile(t)
            # numer half A spill piece (DVE takes the other half)
            sc.wait_ge(s_hA, 1)
            sc.copy(nsp[0:1, 2048:HB], pt[0:1, 2048:4096]).then_inc(s_spA, 1)
            for t in list(tiles_of(4, "A"))[3:]:
                act_tile(t)
            sc.wait_ge(dxr[3], 16)
            for t in list(tiles_of(3, "A"))[:4]:
                act_tile(t)
            sc.wait_ge(s_hB, 1)
            sc.copy(nsp[0:1, HB + 2048 : B], pt[0:1, 2048:4096]).then_inc(s_spB, 1)
            for t in list(tiles_of(3, "A"))[4:]:
                act_tile(t)

        @block.vector
        def _(ve):
            def dve_tile(t):
                nc.vector.scalar_tensor_tensor(
                    dumpV[:, 0:1].to_broadcast((P, DC)), xrm3[:, t, :], 1.0,
                    xrm3[:, t, :], AOP.mult, AOP.mult,
                    accum_out=normsq[:, t : t + 1],
                ).then_inc(s_nsV, 1)

            for c in (0, 1, 2):
                ve.wait_ge(dxr[c], 16)
                for t in tiles_of(c, "V"):
                    dve_tile(t)
            ve.wait_ge(dxr[4], 16)
            tl4 = list(tiles_of(4, "V"))
            for t in tl4[:6]:
                dve_tile(t)
            # numer half A spill piece; ACT takes the other half
            ve.wait_ge(s_hA, 1)
            nc.vector.tensor_copy(nsp[0:1, 0:2048], pt[0:1, 0:2048]).then_inc(s_spA, 1)
            for t in tl4[6:]:
                dve_tile(t)
            ve.wait_ge(dxr[3], 16)
            tl3 = list(tiles_of(3, "V"))
            for t in tl3[:4]:
                dve_tile(t)
            # numer half B spill piece
            ve.wait_ge(s_hB, 1)
            nc.vector.tensor_copy(nsp[0:1, HB : HB + 2048], pt[0:1, 0:2048]).then_inc(s_spB, 1)
            for t in tl3[4:]:
                dve_tile(t)

        @block.gpsimd
        def _(gp):
            gp.memset(one1[:, :], 1.0)

        @block.tensor
        def _(pe):
            # HAM warmup: junk matmuls on garbage SBUF while DMA issues/preamble
            # run; keeps the PE clock gate at full rate for the real stream.
            for _ in range(16):
                nc.tensor.matmul(
                    pt[64:65, 0:256], junkb[0:1, 0:1], junkb[0:1, 0:256],
                    start=True, stop=True,
                )
            # s0: DoubleRow over row-tile pairs -> psum[0:1, 0:512]
            pe.wait_ge(sm0, 16)
            s0_order = [0, 1, 2, 4, 3]
            first = True
            for ci in s0_order:
                lo, hi = CHUNKS[ci][0], CHUNKS[ci][1]
                pe.wait_ge(dxr[ci], 16)
                for t2 in range(lo // 2, hi // 2):
                    mm = nc.tensor.matmul(
                        pt[0:1, 0:DC],
                        m0d[:, :, t2 : t2 + 1],
                        xrm3[:, 2 * t2 : 2 * t2 + 2, :],
                        start=first, stop=(ci == 3 and t2 == hi // 2 - 1),
                        perf_mode=DR,
                    )
                    first = False
            mm.then_inc(s_pe, 1)
            # spread m16row chunks onto partitions: K=1 matmuls vs ones
            pe.wait_ge(s_cast, 1)
            for k in range(NK):
                mm = nc.tensor.matmul(
                    pt[:, 4092 + k : 4093 + k],
                    m16row[0:1, k * P : (k + 1) * P],
                    one1[0:1, 0:1],
                    start=True, stop=True,
                )
            mm.then_inc(s_tr, 1)
            # keep the PE clock warm while waiting for m8pad + xt arrival
            for _ in range(4):
                nc.tensor.matmul(
                    pt[64:65, 0:256], junkb[0:1, 0:1], junkb[0:1, 0:256],
                    start=True, stop=True,
                )
            pe.wait_ge(s_m8, 1)
            # numer: DoubleRow into psum[0:1, :]; half B reuses the same psum
            # row, so it waits until both half A spill pieces are out.
            # g-outer order so the two contraction halves accumulate per column.
            for h in range(2):
                pe.wait_ge(dxt[(h, 0)], 16)
                pe.wait_ge(dxt[(h, 1)], 16)
                if h == 1:
                    pe.wait_ge(s_spA, 2)
                for g in range(2):
                    for j in range(8):
                        mm = nc.tensor.matmul(
                            pt[0:1, j * DC : (j + 1) * DC],
                            m8pad[:, g * 64 : g * 64 + 33 : 32],
                            xt6[:, g, h, j, :, :],
                            start=(g == 0), stop=(g == 1),
                            perf_mode=DR,
                        )
                mm.then_inc(s_hA if h == 0 else s_hB, 1)

    ctx.close()
    return nc


_PROGRAM = None
LAST_RESULT = None  # BassKernelResults of the most recent run (for profiling)


def _host_inputs(labels, datas):
    import ml_dtypes

    fp8 = ml_dtypes.float8_e4m3
    labels = np.asarray(labels, dtype=np.float32)
    datas = np.asarray(datas, dtype=np.float32)

    mask0 = (labels[:, 0] >= labels[:, 1]).astype(np.float32)  # argmax==0
    x8 = datas.astype(fp8)

    # m0 pair layout: [p, a*32+t2] = mask0[(2*t2+a)*128+p]
    mt = mask0.reshape(T, P)
    m0 = np.empty((P, T), dtype=np.float32)
    half = T // 2
    m0[:, 0:half] = mt[0::2].T
    m0[:, half:T] = mt[1::2].T
    m0 = np.ascontiguousarray(m0).astype(fp8)

    in_maps = []
    for c in range(NCORES):
        xc = x8[:, c * DC : (c + 1) * DC]                       # [8192, 512] fp8
        x_rm = np.ascontiguousarray(
            xc.reshape(T, P, DC).transpose(1, 0, 2)).reshape(P, T * DC)
        xt = np.ascontiguousarray(
            xc.T.reshape(2, 2, P, 2, 8, 512).transpose(2, 0, 3, 4, 1, 5)
        ).reshape(P, NK * B)
        in_maps.append({"xrm": x_rm, "xt": xt, "m0": m0})
    return in_maps, mask0


def _host_finish(results, mask0):
    mask0 = mask0.astype(np.float64)
    mask1 = 1.0 - mask0
    n0 = float(mask0.sum())
    n1 = float(mask1.sum())

    numer = np.zeros(B)
    normsq = np.zeros(B)
    msq = 0.0
    for c in range(NCORES):
        r = results[c]
        numer += np.asarray(r["out_num"], dtype=np.float64).reshape(-1)
        normsq += np.asarray(r["out_nrm"], dtype=np.float64).T.reshape(-1)
        m8p = np.asarray(r["out_m8p"]).astype(np.float64)
        for k in range(NK):
            msq += float((m8p[:, k * 32] ** 2).sum())

    if n0 > 0.0:
        xnorm = np.maximum(np.sqrt(normsq), EPS)
        mnorm = max(np.sqrt(msq), EPS * MS * max(n0, 1.0))
        q = np.abs(numer) / (xnorm * mnorm)
        sim = 1.0 - float((mask0 * q).sum()) / n0
        dif = float((mask1 * q).sum()) / n1 if n1 > 0.0 else 0.0
    else:
        sim = 0.0
        dif = 0.0

    sim = np.float32(sim)
    dif = np.float32(dif)
    return (np.float32(sim + dif), sim, dif)


def kernel(labels, datas):
    global _PROGRAM, LAST_RESULT
    from concourse.bass_utils import run_bass_kernel_spmd

    in_maps, mask0 = _host_inputs(labels, datas)
    if _PROGRAM is None:
        _PROGRAM = _build_program()
    res = run_bass_kernel_spmd(_PROGRAM, in_maps, list(range(NCORES)))
    LAST_RESULT = res
    return _host_finish(res.results, mask0)


# revision 38
# speedup vs baseline: 1.3785x; 1.0007x over previous
"""Trainium2 Bass kernel for nn_Diff_Label01_Loss (masked cosine-similarity loss).

Contract: kernel(labels, datas) takes FULL inputs (labels [8192,2] f32,
datas [8192,4096] f32), returns (total_loss, sim_loss, differ_loss).

Strategy — shard D (columns) across the 8 cores; NO collective:
  Core c owns cols [c*512, (c+1)*512) of datas, in TWO fp8 layouts
  (8.4MB/core total):
    x_rm [128, 64, 512]   row-tiles (partition p of tile t = row t*128+p)
    xT   [128, 2, 2, 8, 2, 512]  [p, g, h, j, kt, c] = x[h*4096+j*512+c,
                                  (2g+kt)*128+p] — pair-interleaved for
                                  DoubleRow fp8 matmuls
  s0_c   = masked column sum of the core's slice — PE DoubleRow matmuls,
           mask-pair stationary, accumulated in psum[0:1, 0:512]
  m8_c   = fp8(bf16(s0_c * 2^-6)) — ACT cast to bf16, four K=1 matmuls
           spread it onto partitions, ACT copies into pair-layout slots
  numer  = x_slice @ m8_c — PE DoubleRow into psum[0:1, 0:4096]; half B
           reuses the row after half A's spill (DR matmuls to psum
           partition 32/64 fail ISA checks); [1,2048] spill pieces split
           across DVE and ACT
  normsq = per-row sum of squares — split DVE scalar_tensor_tensor /
           ACT Square activation, 38/26 tiles, chasing the x_rm ingest
           chunks in arrival order (GPSIMD cannot run tensor ops on this
           compiler, and fp8 has no 2x DVE mode, so these two engines are
           the throughput wall of the whole kernel)

  The PE runs junk matmuls on garbage SBUF during the ~10us DMA-issue /
  preamble window so the HAM clock gate is already at full rate when the
  real DoubleRow stream starts; a few more keep it warm across the
  m-dance gap. DMA completion semaphores lag the last data byte by
  several us when both HWDGE rings are busy (per-engine sem-inc
  descriptors pay the write-receipt round trip), which is why the chunk
  schedule leans early.

Host: packs fp8 layouts, then combines per-core partials in f64:
  numer_i = sum_c numer_c[i]; |x_i|^2 = sum_c normsq_c[i];
  |m|^2 = sum_c |m8_c|^2; cos_i = numer_i / (|x_i| |m|) — scale-invariant
  in m, so the 2^-6 scaling and the n0 division drop out.
"""

import contextlib

import numpy as np

B = 8192
D = 4096
P = 128
NCORES = 8
DC = D // NCORES        # 512 cols per core
T = B // P              # 64 row tiles
NK = DC // P            # 4 col chunks
HB = B // 2             # rows per half
MS = 2.0 ** -6          # m scale (keeps s0 in fp8 range)
EPS = 1e-8
# x_rm ingest chunks: tile ranges [lo, hi) and (DVE, ACT) normsq split.
# Chunk 0 is split 4/12 tiles so the vector engines start ~2us earlier.
CHUNKS = [
    (0, 4, (3, 1)),      # 0a: 0.25MB, SP ring first
    (4, 16, (7, 5)),     # 0b: 0.75MB, SP
    (16, 32, (10, 6)),   # 1:  1MB, ACT ring
    (32, 48, (9, 7)),    # 2:  1MB, SP
    (48, 64, (9, 7)),    # 3:  1MB, ACT
]
NV_TOT = sum(c[2][0] for c in CHUNKS)
NA_TOT = sum(c[2][1] for c in CHUNKS)


def _build_program():
    import concourse.bass as bass
    import concourse.mybir as mybir

    f32 = mybir.dt.float32
    bf16 = mybir.dt.bfloat16
    fp8 = mybir.dt.float8e4
    AOP = mybir.AluOpType
    AF = mybir.ActivationFunctionType
    DR = mybir.MatmulPerfMode.DoubleRow

    nc = bass.Bass(trn_type="TRN2", num_devices=NCORES)

    xrm_d = nc.dram_tensor("xrm", [P, T * DC], fp8, kind="ExternalInput")
    xt_d = nc.dram_tensor("xt", [P, NK * B], fp8, kind="ExternalInput")
    m0_d = nc.dram_tensor("m0", [P, T], fp8, kind="ExternalInput")
    out_num = nc.dram_tensor("out_num", [1, B], f32, kind="ExternalOutput")
    out_nrm = nc.dram_tensor("out_nrm", [P, T], f32, kind="ExternalOutput")
    out_m8p = nc.dram_tensor("out_m8p", [P, 128], fp8, kind="ExternalOutput")

    ctx = contextlib.ExitStack()
    sb = lambda name, shape, dt: ctx.enter_context(nc.sbuf_tensor(name, shape, dt))

    x_rm = sb("x_rm", [P, T * DC], fp8)
    xts = sb("xts", [P, NK * B], fp8)
    m0s = sb("m0s", [P, T], fp8)         # [p, a*32+t2] = mask0(row (2*t2+a)*128+p)
    m8pad = sb("m8pad", [P, 128], fp8)   # stationary slots: col k*32 = m[k*128+p]
    dumpV = sb("dumpV", [P, 1], fp8)
    dumpA = sb("dumpA", [P, 1], fp8)
    normsq = sb("normsq", [P, T], f32)
    m16row = sb("m16row", [1, DC], bf16)
    one1 = sb("one1", [1, 1], bf16)
    nsp = sb("nsp", [1, B], f32)         # numer row
    pacc = sb("pacc", [1, 1], f32)       # probe accum scratch (unused output)
    junkb = sb("junkb", [1, 1024], bf16)  # never written; junk warmup reads

    pt = ctx.enter_context(nc.psum_tensor("pt", [P, 4096]))

    sem = lambda name: ctx.enter_context(nc.semaphore(name))
    dxr = [sem(f"dxr{i}") for i in range(len(CHUNKS))]
    dxt = {(h, g): sem(f"dxt{h}{g}") for h in range(2) for g in range(2)}
    sm0 = sem("sm0")
    s_pe = sem("s_pe")
    s_cast = sem("s_cast")
    s_tr = sem("s_tr")
    s_m8 = sem("s_m8")
    s_hA = sem("s_hA")
    s_hB = sem("s_hB")
    s_spA = sem("s_spA")
    s_spB = sem("s_spB")
    s_nsV = sem("s_nsV")
    s_nsA = sem("s_nsA")
    s_out = sem("s_out")

    xrm3 = x_rm.rearrange("p (t c) -> p t c", c=DC)
    xt6 = xts.rearrange("p (g h j k c) -> p g h j k c", g=2, h=2, j=8, k=2)
    m0d = m0s.rearrange("p (a t) -> p a t", a=2)

    def rm_chunk(q):
        lo, hi = CHUNKS[q][0], CHUNKS[q][1]
        return slice(lo * DC, hi * DC)

    def xt_sl(h, g):
        base = (g * 2 + h) * HB * 2
        return slice(base, base + HB * 2)   # full 1MB block

    # tile ownership within a chunk: DVE first, ACT rest
    def tiles_of(c, eng):
        lo, hi, (nv, na) = CHUNKS[c]
        if eng == "V":
            return range(lo, lo + nv)
        return range(lo + nv, hi)

    with nc.Block() as block:

        @block.sync
        def _(sync):
            sync.dma_start(m0s[:, :], m0_d[:, :]).then_inc(sm0, 16)
            for q in (0, 1, 3):
                sl = rm_chunk(q)
                sync.dma_start(x_rm[:, sl], xrm_d[:, sl]).then_inc(dxr[q], 16)
            for (h, g) in ((0, 0), (1, 0)):
                sl = xt_sl(h, g)
                sync.dma_start(xts[:, sl], xt_d[:, sl]).then_inc(dxt[(h, g)], 16)
            # numer half A out as soon as both spill pieces land
            sync.wait_ge(s_spA, 2)
            sync.dma_start(out_num[:, 0:HB], nsp[:, 0:HB]).then_inc(s_out, 16)
            sync.wait_ge(s_spB, 2)
            sync.dma_start(out_num[:, HB:B], nsp[:, HB:B]).then_inc(s_out, 16)
            # normsq out when both engines are done
            sync.wait_ge(s_nsV, NV_TOT)
            sync.wait_ge(s_nsA, NA_TOT)
            sync.dma_start(out_nrm[:, :], normsq[:, :]).then_inc(s_out, 16)
            sync.wait_ge(s_out, 64)

        @block.scalar
        def _(sc):
            for q, sem_ in ((2, dxr[2]), (4, dxr[4])):
                sl = rm_chunk(q)
                sc.dma_start(x_rm[:, sl], xrm_d[:, sl]).then_inc(sem_, 16)
            for (h, g) in ((0, 1), (1, 1)):
                sl = xt_sl(h, g)
                sc.dma_start(xts[:, sl], xt_d[:, sl]).then_inc(dxt[(h, g)], 16)

            def act_tile(t):
                sc.activation(dumpA[:, 0:1].to_broadcast((P, DC)), xrm3[:, t, :],
                              AF.Square,
                              accum_out=normsq[:, t : t + 1]).then_inc(s_nsA, 1)

            # 11 tiles before the cast (lands ~when s0 completes)
            sc.wait_ge(dxr[0], 16)
            for t in tiles_of(0, "A"):
                act_tile(t)
            sc.wait_ge(dxr[1], 16)
            for t in tiles_of(1, "A"):
                act_tile(t)
            sc.wait_ge(dxr[2], 16)
            for t in list(tiles_of(2, "A"))[:5]:
                act_tile(t)
            # m dance: cast s0 -> bf16 row; after PE spreads it, pack fp8 slots
            sc.wait_ge(s_pe, 1)
            sc.activation(m16row[:, :], pt[0:1, 0:DC], AF.Copy, scale=MS).then_inc(s_cast, 1)
            sc.wait_ge(s_tr, 1)
            sc.copy(m8pad[:, 0:97:32], pt[:, 4092:4096]).then_inc(s_m8, 1)
            sc.dma_start(out_m8p[:, :], m8pad[:, :]).then_inc(s_out, 16)
            for t in list(tiles_of(2, "A"))[5:]:
                act_tile(t)
            sc.wait_ge(dxr[4], 16)
            for t in list(tiles_of(4, "A"))[:3]:
                act_tile(t)
            # numer half A spill piece (DVE takes the other half)
            sc.wait_ge(s_hA, 1)
            sc.copy(nsp[0:1, 2048:HB], pt[0:1, 2048:4096]).then_inc(s_spA, 1)
            for t in list(tiles_of(4, "A"))[3:]:
                act_tile(t)
            sc.wait_ge(dxr[3], 16)
            for t in list(tiles_of(3, "A"))[:4]:
                act_tile(t)
            sc.wait_ge(s_hB, 1)
            sc.copy(nsp[0:1, HB + 2048 : B], pt[0:1, 2048:4096]).then_inc(s_spB, 1)
            for t in list(tiles_of(3, "A"))[4:]:
                act_tile(t)

        @block.vector
        def _(ve):
            def dve_tile(t):
                nc.vector.scalar_tensor_tensor(
                    dumpV[:, 0:1].to_broadcast((P, DC)), xrm3[:, t, :], 1.0,
                    xrm3[:, t, :], AOP.mult, AOP.mult,
                    accum_out=normsq[:, t : t + 1],
                ).then_inc(s_nsV, 1)

            for c in (0, 1, 2):
                ve.wait_ge(dxr[c], 16)
                for t in tiles_of(c, "V"):
                    dve_tile(t)
            ve.wait_ge(dxr[4], 16)
            tl4 = list(tiles_of(4, "V"))
            for t in tl4[:6]:
                dve_tile(t)
            # numer half A spill piece; ACT takes the other half
            ve.wait_ge(s_hA, 1)
            nc.vector.tensor_copy(nsp[0:1, 0:2048], pt[0:1, 0:2048]).then_inc(s_spA, 1)
            for t in tl4[6:]:
                dve_tile(t)
            ve.wait_ge(dxr[3], 16)
            tl3 = list(tiles_of(3, "V"))
            for t in tl3[:4]:
                dve_tile(t)
            # numer half B spill piece
            ve.wait_ge(s_hB, 1)
            nc.vector.tensor_copy(nsp[0:1, HB : HB + 2048], pt[0:1, 0:2048]).then_inc(s_spB, 1)
            for t in tl3[4:]:
                dve_tile(t)
            # --- perf probes (timing only; outputs unused) ---
            for _ in range(2):
                nc.vector.tensor_scalar(
                    dumpV[0:1, 0:1].to_broadcast((1, DC)), junkb[0:1, 0:DC],
                    1.0, 1.0, AOP.mult, AOP.mult, accum_out=pacc[:, :])
            for _ in range(2):
                nc.vector.tensor_reduce(
                    pacc[:, :], junkb[0:1, 0:DC],
                    axis=mybir.AxisListType.X, op=AOP.add)

        @block.gpsimd
        def _(gp):
            gp.memset(one1[:, :], 1.0)

        @block.tensor
        def _(pe):
            # HAM warmup: junk matmuls on garbage SBUF while DMA issues/preamble
            # run; keeps the PE clock gate at full rate for the real stream.
            for _ in range(16):
                nc.tensor.matmul(
                    pt[64:65, 0:256], junkb[0:1, 0:1], junkb[0:1, 0:256],
                    start=True, stop=True,
                )
            # s0: DoubleRow over row-tile pairs -> psum[0:1, 0:512]
            pe.wait_ge(sm0, 16)
            s0_order = [0, 1, 2, 4, 3]
            first = True
            for ci in s0_order:
                lo, hi = CHUNKS[ci][0], CHUNKS[ci][1]
                pe.wait_ge(dxr[ci], 16)
                for t2 in range(lo // 2, hi // 2):
                    mm = nc.tensor.matmul(
                        pt[0:1, 0:DC],
                        m0d[:, :, t2 : t2 + 1],
                        xrm3[:, 2 * t2 : 2 * t2 + 2, :],
                        start=first, stop=(ci == 3 and t2 == hi // 2 - 1),
                        perf_mode=DR,
                    )
                    first = False
            mm.then_inc(s_pe, 1)
            # spread m16row chunks onto partitions: K=1 matmuls vs ones
            pe.wait_ge(s_cast, 1)
            for k in range(NK):
                mm = nc.tensor.matmul(
                    pt[:, 4092 + k : 4093 + k],
                    m16row[0:1, k * P : (k + 1) * P],
                    one1[0:1, 0:1],
                    start=True, stop=True,
                )
            mm.then_inc(s_tr, 1)
            # keep the PE clock warm while waiting for m8pad + xt arrival
            for _ in range(4):
                nc.tensor.matmul(
                    pt[64:65, 0:256], junkb[0:1, 0:1], junkb[0:1, 0:256],
                    start=True, stop=True,
                )
            pe.wait_ge(s_m8, 1)
            # numer: DoubleRow into psum[0:1, :]; half B reuses the same psum
            # row, so it waits until both half A spill pieces are out.
            # g-outer order so the two contraction halves accumulate per column.
            for h in range(2):
                pe.wait_ge(dxt[(h, 0)], 16)
                pe.wait_ge(dxt[(h, 1)], 16)
                if h == 1:
                    pe.wait_ge(s_spA, 2)
                for g in range(2):
                    for j in range(8):
                        mm = nc.tensor.matmul(
                            pt[0:1, j * DC : (j + 1) * DC],
                            m8pad[:, g * 64 : g * 64 + 33 : 32],
                            xt6[:, g, h, j, :, :],
                            start=(g == 0), stop=(g == 1),
                            perf_mode=DR,
                        )
                mm.then_inc(s_hA if h == 0 else s_hB, 1)

    ctx.close()
    return nc


_PROGRAM = None
LAST_RESULT = None  # BassKernelResults of the most recent run (for profiling)


def _host_inputs(labels, datas):
    import ml_dtypes

    fp8 = ml_dtypes.float8_e4m3
    labels = np.asarray(labels, dtype=np.float32)
    datas = np.asarray(datas, dtype=np.float32)

    mask0 = (labels[:, 0] >= labels[:, 1]).astype(np.float32)  # argmax==0
    x8 = datas.astype(fp8)

    # m0 pair layout: [p, a*32+t2] = mask0[(2*t2+a)*128+p]
    mt = mask0.reshape(T, P)
    m0 = np.empty((P, T), dtype=np.float32)
    half = T // 2
    m0[:, 0:half] = mt[0::2].T
    m0[:, half:T] = mt[1::2].T
    m0 = np.ascontiguousarray(m0).astype(fp8)

    in_maps = []
    for c in range(NCORES):
        xc = x8[:, c * DC : (c + 1) * DC]                       # [8192, 512] fp8
        x_rm = np.ascontiguousarray(
            xc.reshape(T, P, DC).transpose(1, 0, 2)).reshape(P, T * DC)
        xt = np.ascontiguousarray(
            xc.T.reshape(2, 2, P, 2, 8, 512).transpose(2, 0, 3, 4, 1, 5)
        ).reshape(P, NK * B)
        in_maps.append({"xrm": x_rm, "xt": xt, "m0": m0})
    return in_maps, mask0


def _host_finish(results, mask0):
    mask0 = mask0.astype(np.float64)
    mask1 = 1.0 - mask0
    n0 = float(mask0.sum())
    n1 = float(mask1.sum())

    numer = np.zeros(B)
    normsq = np.zeros(B)
    msq = 0.0
    for c in range(NCORES):
        r = results[c]
        numer += np.asarray(r["out_num"], dtype=np.float64).reshape(-1)
        normsq += np.asarray(r["out_nrm"], dtype=np.float64).T.reshape(-1)
        m8p = np.asarray(r["out_m8p"]).astype(np.float64)
        for k in range(NK):
            msq += float((m8p[:, k * 32] ** 2).sum())

    if n0 > 0.0:
        xnorm = np.maximum(np.sqrt(normsq), EPS)
        mnorm = max(np.sqrt(msq), EPS * MS * max(n0, 1.0))
        q = np.abs(numer) / (xnorm * mnorm)
        sim = 1.0 - float((mask0 * q).sum()) / n0
        dif = float((mask1 * q).sum()) / n1 if n1 > 0.0 else 0.0
    else:
        sim = 0.0
        dif = 0.0

    sim = np.float32(sim)
    dif = np.float32(dif)
    return (np.float32(sim + dif), sim, dif)


def kernel(labels, datas):
    global _PROGRAM, LAST_RESULT
    from concourse.bass_utils import run_bass_kernel_spmd

    in_maps, mask0 = _host_inputs(labels, datas)
    if _PROGRAM is None:
        _PROGRAM = _build_program()
    res = run_bass_kernel_spmd(_PROGRAM, in_maps, list(range(NCORES)))
    LAST_RESULT = res
    return _host_finish(res.results, mask0)
